# revision 3
# baseline (speedup 1.0000x reference)
"""AtIndexPooler (embedding lookup) on 8 TRN2 NeuronCores.

Data-parallel along batch: each core owns B/8 = 64 batch rows. Per core the
hidden_state shard is viewed as a flat row table [64*512, 1024] with the two
missing-embedding rows appended at the end ([32770, 1024] total). The host
turns indices into flat row offsets (invalid index -1 -> appended missing
row); the device performs the lookup as two half-width 128-row indirect DMA
gathers (2KB per SBUF partition each) pipelined with two half-width stores on
the two HWDGE rings.

Design notes (from HW traces of previous iterations):
- The indirect offset table must be [128, 1] int32 (one per partition). A
  plain HBM->SBUF load of that layout sprays 128 4-byte descriptors (~1us).
  Instead the host packs offsets as lo/hi int16 in a [16, 128] tile and the
  kernel loads it with the xbar DMA transpose (one 16x128 int16 tile = one
  descriptor); the transposed [128, 16] tile bitcast to int32 gives the
  [128, 1] table.
- The gather is split along hidden into two element_offset chunks sharing one
  offset table (offsets are premultiplied by 2 because the indirect's address
  coefficient comes from the sliced AP shape, 512, not the row stride, 1024).
  Each chunk spans all 128 partitions: partial-partition indirects are a
  known device-wedging hazard.
- The two stores go on the two HWDGE rings (SP + ACT) gated on their chunk's
  gather semaphore, overlapping store 0 with gather 1.
- Bass.__init__'s const-AP memsets + init all-engine barrier are deleted from
  the IR (nothing reads the consts; all DMAs are semaphore-gated; NRT
  serializes executions). The per-engine drains are kept: removing them made
  a 1.4us drain reappear on the Pool engine's critical path.
- enable_partition_id=False / monotonic_sem_count=0 drop unused prologue work.
"""

import sys

import numpy as np

if "/opt/trn_rl_repo" not in sys.path:
    sys.path.insert(0, "/opt/trn_rl_repo")

from concourse import bacc, bass, mybir
from concourse.bass_utils import run_bass_kernel_spmd

BATCH, SEQ_LEN, HIDDEN = 512, 512, 1024
NUM_INDICES = 2
N_CORES = 8
B_SHARD = BATCH // N_CORES                # 64 batches per core
ROWS = B_SHARD * NUM_INDICES              # 128 gather rows = 128 partitions
DATA_ROWS = B_SHARD * SEQ_LEN + NUM_INDICES  # 32770 rows in the lookup table
HALF_H = HIDDEN // 2

_NC_CACHE = None
LAST_RESULT = None  # BassKernelResults of the most recent run (for profiling)


def _strip_init_preamble(nc):
    """Remove the const-AP memsets and the init all-engine barrier emitted by
    Bass.__init__ (keep the drains — see module docstring)."""
    blk = nc.main_func.blocks[0]
    drop = [
        i
        for i in blk.instructions
        if isinstance(i, mybir.InstMemset)
        or (isinstance(i, mybir.InstEventSemaphore) and i.name.startswith("barrier_"))
    ]
    for i in drop:
        blk.instructions.remove(i)
        nc.inst_map.pop(i.name, None)


def _build_nc():
    nc = bacc.Bacc(
        "TRN2",
        target_bir_lowering=False,
        debug=False,
        num_devices=N_CORES,
        enable_partition_id=False,
        monotonic_sem_count=0,
    )
    data = nc.dram_tensor("data", [DATA_ROWS, HIDDEN], mybir.dt.float32, kind="ExternalInput")
    offs = nc.dram_tensor("offs", [16, ROWS], mybir.dt.int16, kind="ExternalInput")
    out = nc.dram_tensor("out", [ROWS, HIDDEN], mybir.dt.float32, kind="ExternalOutput")

    sA = nc.alloc_semaphore("sA")     # offs transpose-load completion
    sB0 = nc.alloc_semaphore("sB0")   # gather chunk 0 completion
    sB1 = nc.alloc_semaphore("sB1")   # gather chunk 1 completion
    sC0 = nc.alloc_semaphore("sC0")   # store chunk 0 completion
    sC1 = nc.alloc_semaphore("sC1")   # store chunk 1 completion
    offs_t = nc.alloc_sbuf_tensor("offs_t", [ROWS, 16], mybir.dt.int16)
    gath = nc.alloc_sbuf_tensor("gath", [ROWS, HIDDEN], mybir.dt.float32)

    _strip_init_preamble(nc)

    # offs[r, p] = int16 lane r of offset[p] (r=0 lo, r=1 hi, rest zero).
    # One 16x128 xbar tile -> offs_t[p, r] = offs[r, p].
    nc.sync.dma_start(out=offs_t[:, :], in_=offs[:, :], transpose=True).then_inc(sA, 16)

    off_tab = offs_t[:, 0:2].bitcast(mybir.dt.int32)  # [128, 1] int32
    half = data[:, 0:HALF_H]  # ap [[1024, 32770], [1, 512]], offset 0
    nc.gpsimd.wait_ge(sA, 16)
    nc.gpsimd.indirect_dma_start(
        out=gath[:, 0:HALF_H],
        out_offset=None,
        in_=half,
        in_offset=bass.IndirectOffsetOnAxis(ap=off_tab, axis=0),
        element_offset=0,
    ).then_inc(sB0, 16)
    nc.gpsimd.indirect_dma_start(
        out=gath[:, HALF_H:],
        out_offset=None,
        in_=half,
        in_offset=bass.IndirectOffsetOnAxis(ap=off_tab, axis=0),
        element_offset=HALF_H,
    ).then_inc(sB1, 16)

    nc.sync.wait_ge(sB0, 16)
    nc.sync.dma_start(out=out[:, 0:HALF_H], in_=gath[:, 0:HALF_H]).then_inc(sC0, 16)
    nc.scalar.wait_ge(sB1, 16)
    nc.scalar.dma_start(out=out[:, HALF_H:], in_=gath[:, HALF_H:]).then_inc(sC1, 16)

    nc.sync.wait_ge(sC0, 16)
    nc.sync.wait_ge(sC1, 16)
    nums = sorted(s.num for s in (sA, sB0, sB1, sC0, sC1))
    assert nums == list(range(nums[0], nums[0] + 5))
    nc.sync.sem_clear(range(nums[0], nums[-1] + 1))

    nc.compile()
    return nc


def kernel(hidden_state, missing_embeddings, indices):
    global _NC_CACHE, LAST_RESULT
    hidden_state = np.ascontiguousarray(np.asarray(hidden_state, dtype=np.float32))
    missing_embeddings = np.ascontiguousarray(np.asarray(missing_embeddings, dtype=np.float32))
    indices = np.asarray(indices)

    if _NC_CACHE is None:
        _NC_CACHE = _build_nc()
    nc = _NC_CACHE

    base = (np.arange(B_SHARD, dtype=np.int64) * SEQ_LEN)[:, None]
    miss_rows = B_SHARD * SEQ_LEN + np.arange(NUM_INDICES, dtype=np.int64)[None, :]
    in_maps = []
    for c in range(N_CORES):
        hs = hidden_state[c * B_SHARD : (c + 1) * B_SHARD].reshape(B_SHARD * SEQ_LEN, HIDDEN)
        idx = indices[c * B_SHARD : (c + 1) * B_SHARD].astype(np.int64)  # [64, 2]
        flat = np.where(idx >= 0, base + np.clip(idx, 0, SEQ_LEN - 1), miss_rows).reshape(ROWS)
        data = np.concatenate([hs, missing_embeddings], axis=0)
        # The indirect's address coefficient is the sliced-AP row length
        # (HALF_H), so premultiply: value * HALF_H == flat * HIDDEN.
        off32 = (flat * 2).astype(np.uint32)
        offs = np.zeros((16, ROWS), dtype=np.uint16)
        offs[0] = (off32 & 0xFFFF).astype(np.uint16)
        offs[1] = (off32 >> 16).astype(np.uint16)
        in_maps.append({"data": data, "offs": offs.view(np.int16)})

    LAST_RESULT = run_bass_kernel_spmd(nc, in_maps, core_ids=list(range(N_CORES)))
    outs = [
        LAST_RESULT.results[c]["out"].reshape(B_SHARD, NUM_INDICES * HIDDEN)
        for c in range(N_CORES)
    ]
    return np.concatenate(outs, axis=0)
